# revision 10
# baseline (speedup 1.0000x reference)
"""Causal self-attention (B=4, S=2048, C=1024, H=16) on 8 TRN2 cores.

Sharding: core = (batch b = core//2, head-group g = core%2). Each core
computes q/k/v projections for its 8 heads, causal attention, and a
partial output projection; the host sums the two per-batch partials and
adds the (host-computed) bias vector bp + Wp @ bv.

On-chip layout is transpose-free: the host passes tgt[b].T and the
weight slices pre-transposed, so every matmul operand already has the
contraction dim on partitions. All matmuls run as float32r (measured
~1e-5 rel-rms on HW, full rate at N>=256).

Softmax runs without max-subtraction (scores are bounded ~|20| for this
problem's 0.02-scale weights; fp32 exp is safe to ~88). Causality is
enforced with an additive -1e10 block/triangle mask fused into the
score PSUM; the denominator comes from a ones-column appended to V so
the AV matmul yields [y; sum(p)] in one pass.
"""
import numpy as np

import concourse.bass as bass
import concourse.mybir as mybir
import concourse.tile as tile
from concourse.bass_utils import run_bass_kernel_spmd

dt = mybir.dt
F = mybir.ActivationFunctionType
Alu = mybir.AluOpType

B, S, C, H = 4, 2048, 1024, 16
D = C // H            # 64 head dim
GC = C // 2           # 512 channels per head-group (8 heads)
NPAIR = 4             # head pairs per core
NCH = S // 512        # 4 query chunks
NSB = S // 128        # 16 s blocks
NCI = C // 128        # 8 contraction blocks
SCALE = 0.125         # 1/sqrt(D)
NEG = -1.0e10

_nc_cache = {}


def _split_multi_waits(nc):
    """This container's walrus accepts at most ONE sem wait per
    instruction ("Too many sync wait commands"). Hoist extra waits onto
    NOPs inserted just before the instruction on the same engine."""
    n = 0
    for fn in nc.m.functions:
        for bb in fn.blocks:
            new = []
            dirty = False
            for inst in bb.instructions:
                si = inst.sync_info
                if si is not None and si.on_wait and len(si.on_wait) > 1:
                    waits = list(si.on_wait)
                    for j, w in enumerate(waits[1:]):
                        new.append(mybir.InstNoOp(
                            name=f"{inst.name}-wsplit{j}",
                            sync_info=mybir.SyncInfo(on_wait=[w], on_update=[]),
                            engine=inst.engine,
                            bass_nofuse=True,
                        ))
                        n += 1
                    si.on_wait = waits[:1]
                    dirty = True
                new.append(inst)
            if dirty:
                bb.instructions = new
    return n


def _build(with_mask, split=True):
    nc = bass.Bass("TRN2")
    f32r = dt.float32r
    f32 = dt.float32

    xt = nc.dram_tensor("xt", [C, S], f32r, kind="ExternalInput")
    wqt = nc.dram_tensor("wqt", [C, GC], f32r, kind="ExternalInput")
    wkt = nc.dram_tensor("wkt", [C, GC], f32r, kind="ExternalInput")
    wvt = nc.dram_tensor("wvt", [C, GC], f32r, kind="ExternalInput")
    wpt = nc.dram_tensor("wpt", [GC, C], f32r, kind="ExternalInput")
    bqd = nc.dram_tensor("bqd", [GC], f32, kind="ExternalInput")
    bkd = nc.dram_tensor("bkd", [GC], f32, kind="ExternalInput")
    onesd = nc.dram_tensor("onesd", [1, D], f32r, kind="ExternalInput")
    vonesd = nc.dram_tensor("vonesd", [128, NSB * 8], f32r, kind="ExternalInput")
    dmaskd = nc.dram_tensor("dmaskd", [4, 128, 512], f32, kind="ExternalInput")
    if with_mask:
        masktd = nc.dram_tensor("masktd", [S, S], f32, kind="ExternalInput")
    outd = nc.dram_tensor("out", [S, C], f32, kind="ExternalOutput")

    with tile.TileContext(nc) as tc:
        with tc.tile_pool(name="persist", bufs=1) as persist:
            qt_sb = [persist.tile([128, S], f32r, tag=f"qt{t}", name=f"qt{t}")
                     for t in range(NPAIR)]
            kt_sb = [persist.tile([128, S], f32r, tag=f"kt{t}", name=f"kt{t}")
                     for t in range(NPAIR)]
            v_sb = persist.tile([128, NSB * 520], f32r, tag="v")
            ones_sb = persist.tile([1, D], f32r, tag="ones")
            nc.sync.dma_start(out=ones_sb, in_=onesd[:, :])
            bq_sb = persist.tile([128, 4], f32, tag="bq")
            nc.sync.dma_start(out=bq_sb, in_=bqd.rearrange("(t p) -> p t", p=128))
            bk_sb = persist.tile([128, 4], f32, tag="bk")
            nc.sync.dma_start(out=bk_sb, in_=bkd.rearrange("(t p) -> p t", p=128))
            # ones columns of v (position 64 of each head's 65-col slot)
            nc.sync.dma_start(
                out=v_sb.rearrange("p (s h d) -> p s h d", h=8, d=65)[:, :, :, 64:65],
                in_=vonesd.rearrange("p (s h) -> p s h", h=8).unsqueeze(3),
            )

            # ---------------- phase 1: projections ----------------
            with tc.tile_pool(name="xtp", bufs=1) as xtp, \
                 tc.tile_pool(name="wqk", bufs=2) as wqk, \
                 tc.tile_pool(name="wvp", bufs=1) as wvp, \
                 tc.tile_pool(name="pp", bufs=6, space="PSUM") as pp:
                xt_t = xtp.tile([128, NCI, S], f32r)
                nc.sync.dma_start(out=xt_t, in_=xt.rearrange("(k p) s -> p k s", p=128))

                for t in range(NPAIR):
                    for name, wsrc, dst, bias in (
                        ("q", wqt, qt_sb[t], bq_sb),
                        ("k", wkt, kt_sb[t], bk_sb),
                    ):
                        w_t = wqk.tile([128, NCI, 128], f32r, tag="w")
                        nc.sync.dma_start(
                            out=w_t,
                            in_=wsrc[:, 128 * t:128 * t + 128]
                                .rearrange("(k p) c -> p k c", p=128))
                        for ch in range(NCH):
                            ps = pp.tile([128, 512], dt.float32, tag="ps")
                            for ci in range(NCI):
                                nc.tensor.matmul(
                                    ps, w_t[:, ci, :],
                                    xt_t[:, ci, 512 * ch:512 * ch + 512],
                                    start=(ci == 0), stop=(ci == NCI - 1))
                            if name == "q":
                                nc.vector.tensor_scalar(
                                    dst[:, 512 * ch:512 * ch + 512], ps,
                                    SCALE, bias[:, t:t + 1],
                                    op0=Alu.mult, op1=Alu.add)
                            else:
                                nc.vector.tensor_scalar(
                                    dst[:, 512 * ch:512 * ch + 512], ps,
                                    bias[:, t:t + 1], None, op0=Alu.add)

                wv_t = wvp.tile([128, NCI, GC], f32r)
                nc.sync.dma_start(out=wv_t, in_=wvt.rearrange("(k p) c -> p k c", p=128))
                v3 = v_sb.rearrange("p (s h) -> p s h", h=520)
                for sb in range(NSB):
                    ps = pp.tile([128, GC], dt.float32, tag="ps")
                    for ci in range(NCI):
                        nc.tensor.matmul(
                            ps, xt_t[:, ci, 128 * sb:128 * sb + 128],
                            wv_t[:, ci, :],
                            start=(ci == 0), stop=(ci == NCI - 1))
                    nc.vector.tensor_copy(
                        out=v3[:, sb, :].rearrange("p (h d) -> p h d", d=65)[:, :, 0:64],
                        in_=ps.rearrange("p (h d) -> p h d", d=64))

            # ---------------- phases 2+3 ----------------
            with tc.tile_pool(name="ph23", bufs=1) as ph23, \
                 tc.tile_pool(name="mstr", bufs=3) as mstr, \
                 tc.tile_pool(name="pt", bufs=3) as pt, \
                 tc.tile_pool(name="nrm", bufs=2) as nrm:
                y_sb = [ph23.tile([128, S], f32r, tag=f"y{t}", name=f"y{t}")
                        for t in range(NPAIR)]
                dm_sb = ph23.tile([128, 4, 512], dt.float32, tag="dm")
                nc.sync.dma_start(out=dm_sb, in_=dmaskd.rearrange("o p j -> p o j"))

                # phase 2: attention
                with tc.tile_pool(name="ps2", bufs=2, space="PSUM") as ps2, \
                     tc.tile_pool(name="py2", bufs=2, space="PSUM") as py2, \
                     tc.tile_pool(name="pr2", bufs=2, space="PSUM") as pr2:
                    for t in range(NPAIR):
                        for ch in range(NCH):
                            nkb = 4 * (ch + 1)
                            py = [py2.tile([65, 512], dt.float32, tag="py", name=f"py{_h}")
                                  for _h in range(2)]
                            for kb in range(nkb):
                                ps = ps2.tile([128, 1024], dt.float32, tag="sc")
                                for h in range(2):
                                    nc.tensor.matmul(
                                        ps[:, 512 * h:512 * h + 512],
                                        kt_sb[t][64 * h:64 * h + 64,
                                                 128 * kb:128 * kb + 128],
                                        qt_sb[t][64 * h:64 * h + 64,
                                                 512 * ch:512 * ch + 512],
                                        start=True, stop=True,
                                        tile_position=(64 * h, 0))
                                if kb >= 4 * ch:
                                    o = kb - 4 * ch
                                    for h in range(2):
                                        nc.vector.tensor_tensor(
                                            out=ps[:, 512 * h:512 * h + 512],
                                            in0=ps[:, 512 * h:512 * h + 512],
                                            in1=dm_sb[:, o, :], op=Alu.add)
                                if with_mask:
                                    mt = mstr.tile([128, 512], dt.float32, tag="mt")
                                    nc.sync.dma_start(
                                        out=mt,
                                        in_=masktd[128 * kb:128 * kb + 128,
                                                   512 * ch:512 * ch + 512])
                                    for h in range(2):
                                        nc.vector.tensor_tensor(
                                            out=ps[:, 512 * h:512 * h + 512],
                                            in0=ps[:, 512 * h:512 * h + 512],
                                            in1=mt, op=Alu.add)
                                p_sb = pt.tile([128, 1024], f32r, tag="p")
                                nc.scalar.activation(p_sb, ps, F.Exp)
                                for h in range(2):
                                    nc.tensor.matmul(
                                        py[h],
                                        v_sb[:, 520 * kb + 65 * (2 * t + h):
                                             520 * kb + 65 * (2 * t + h) + 65],
                                        p_sb[:, 512 * h:512 * h + 512],
                                        start=(kb == 0), stop=(kb == nkb - 1))
                            # normalize: y = py[0:64] * broadcast(1/py[64])
                            for h in range(2):
                                rec = nrm.tile([1, 512], f32r, tag="rec")
                                with nc.allow_low_precision(reason="fp32r recip"):
                                    nc.vector.reciprocal(rec, py[h][64:65, :])
                                rep = pr2.tile([64, 512], dt.float32, tag="rep")
                                nc.tensor.matmul(rep, ones_sb, rec,
                                                 start=True, stop=True)
                                rep_sb = nrm.tile([64, 512], dt.float32, tag="repsb")
                                nc.vector.tensor_copy(rep_sb, rep)
                                if h == 0:
                                    nc.vector.tensor_tensor(
                                        out=y_sb[t][0:64, 512 * ch:512 * ch + 512],
                                        in0=py[h][0:64, :], in1=rep_sb, op=Alu.mult)
                                else:
                                    ytmp = nrm.tile([64, 512], f32r, tag="ytmp")
                                    nc.vector.tensor_tensor(
                                        out=ytmp, in0=py[h][0:64, :],
                                        in1=rep_sb, op=Alu.mult)
                                    nc.sync.dma_start(
                                        out=y_sb[t][64:128, 512 * ch:512 * ch + 512],
                                        in_=ytmp)

                # phase 3: output projection
                with tc.tile_pool(name="wpp", bufs=1) as wpp, \
                     tc.tile_pool(name="ost", bufs=4) as ost, \
                     tc.tile_pool(name="po3", bufs=4, space="PSUM") as po3:
                    wp_t = wpp.tile([128, NPAIR, C], f32r)
                    nc.sync.dma_start(out=wp_t,
                                      in_=wpt.rearrange("(t p) c -> p t c", p=128))
                    for sb in range(NSB):
                        for j in range(2):
                            po = po3.tile([128, 512], dt.float32, tag="po")
                            for t in range(NPAIR):
                                # K=128: the pair's two heads stack to a full
                                # contraction, no row-split needed (row-split
                                # accumulation groups hang this HW/compiler).
                                nc.tensor.matmul(
                                    po,
                                    y_sb[t][:, 128 * sb:128 * sb + 128],
                                    wp_t[:, t, 512 * j:512 * j + 512],
                                    start=(t == 0),
                                    stop=(t == NPAIR - 1))
                            o_sb = ost.tile([128, 512], dt.float32, tag="o")
                            nc.vector.tensor_copy(o_sb, po)
                            nc.sync.dma_start(
                                out=outd[128 * sb:128 * sb + 128,
                                         512 * j:512 * j + 512],
                                in_=o_sb)

    if split:
        _split_multi_waits(nc)
    return nc


def _get_nc(with_mask):
    if with_mask not in _nc_cache:
        _nc_cache[with_mask] = _build(with_mask)
    return _nc_cache[with_mask]


def _host_consts():
    ones = np.ones((1, D), np.float32)
    vones = np.ones((128, NSB * 8), np.float32)
    dm = np.zeros((4, 128, 512), np.float32)
    for o in range(4):
        dm[o, :, : 128 * o] = NEG
        blk = dm[o, :, 128 * o: 128 * o + 128]
        i, j = np.meshgrid(np.arange(128), np.arange(128), indexing="ij")
        blk[j < i] = NEG
    return ones, vones, dm


def make_in_maps(tgt, pad_mask, Wq, bq, Wk, bk, Wv, bv, Wp, bp, with_mask):
    ones, vones, dm = _host_consts()
    in_maps = []
    for core in range(8):
        b, g = core // 2, core % 2
        rows = slice(GC * g, GC * g + GC)
        im = {
            "xt": np.ascontiguousarray(tgt[b].T),
            "wqt": np.ascontiguousarray(Wq[rows].T),
            "wkt": np.ascontiguousarray(Wk[rows].T),
            "wvt": np.ascontiguousarray(Wv[rows].T),
            "wpt": np.ascontiguousarray(Wp[:, rows].T),
            "bqd": np.ascontiguousarray(bq[rows] * SCALE),
            "bkd": np.ascontiguousarray(bk[rows]),
            "onesd": ones,
            "vonesd": vones,
            "dmaskd": dm,
        }
        if with_mask:
            im["masktd"] = np.ascontiguousarray(pad_mask[b].T)
        in_maps.append(im)
    return in_maps


def run(tgt, pad_mask, Wq, bq, Wk, bk, Wv, bv, Wp, bp, **spmd_kwargs):
    args = [np.asarray(a, np.float32) for a in
            (tgt, pad_mask, Wq, bq, Wk, bk, Wv, bv, Wp, bp)]
    tgt, pad_mask, Wq, bq, Wk, bk, Wv, bv, Wp, bp = args
    with_mask = bool(np.any(pad_mask))
    nc = _get_nc(with_mask)
    in_maps = make_in_maps(tgt, pad_mask, Wq, bq, Wk, bk, Wv, bv, Wp, bp,
                           with_mask)
    res = run_bass_kernel_spmd(nc, in_maps, core_ids=list(range(8)),
                               **spmd_kwargs)
    bias_vec = (bp + Wp @ bv).astype(np.float32)
    out = np.empty((B, S, C), np.float32)
    for b in range(B):
        out[b] = (res.results[2 * b]["out"] + res.results[2 * b + 1]["out"]
                  + bias_vec)
    return out, res


def kernel(tgt, pad_mask, Wq, bq, Wk, bk, Wv, bv, Wp, bp):
    out, _ = run(tgt, pad_mask, Wq, bq, Wk, bk, Wv, bv, Wp, bp)
    return out


# revision 17
# speedup vs baseline: 18.5545x; 18.5545x over previous
"""Causal self-attention (B=4, S=2048, C=1024, H=16) on 8 TRN2 cores.

Sharding: core = (batch b = core//2, head-group g = core%2). Each core
computes q/k/v projections for its 8 heads, causal attention, and a
partial output projection; the host sums the two per-batch partials and
adds the (host-computed) bias vector bp + Wp @ bv.

On-chip layout is transpose-free: the host passes tgt[b].T and the
weight slices pre-transposed, so every matmul operand already has the
contraction dim on partitions. All matmuls run as float32r (measured
~1e-5 rel-rms on HW, full rate at N>=256).

Softmax runs without max-subtraction (scores are bounded ~|20| for this
problem's 0.02-scale weights; fp32 exp is safe to ~88). Causality is
enforced with an additive -1e10 block/triangle mask fused into the
score PSUM; the denominator comes from a ones-column appended to V so
the AV matmul yields [y; sum(p)] in one pass.
"""
import numpy as np

import concourse.bass as bass
import concourse.mybir as mybir
import concourse.tile as tile
from concourse.bass_utils import run_bass_kernel_spmd

dt = mybir.dt
F = mybir.ActivationFunctionType
Alu = mybir.AluOpType

B, S, C, H = 4, 2048, 1024, 16
D = C // H            # 64 head dim
GC = C // 2           # 512 channels per head-group (8 heads)
NPAIR = 4             # head pairs per core
NCH = S // 512        # 4 query chunks
NSB = S // 128        # 16 s blocks
NCI = C // 128        # 8 contraction blocks
SCALE = 0.125         # 1/sqrt(D)
NEG = -1.0e10

_nc_cache = {}

_DEFAULT_CFG = {
    'pt_bufs': 4,
    'sc_bufs': 2,
    'py_bufs': 2,
    'rep_bufs': 1,
    'pp_bufs': 6,
    'wqk_bufs': 2,
    'ost_bufs': 2,
    'po_bufs': 1,
    'nrm_bufs': 2,
}


def _split_multi_waits(nc):
    """This container's walrus accepts at most ONE sem wait per
    instruction ("Too many sync wait commands"). Hoist extra waits onto
    NOPs inserted just before the instruction on the same engine."""
    n = 0
    for fn in nc.m.functions:
        for bb in fn.blocks:
            new = []
            dirty = False
            for inst in bb.instructions:
                si = inst.sync_info
                if si is not None and si.on_wait and len(si.on_wait) > 1:
                    waits = list(si.on_wait)
                    for j, w in enumerate(waits[1:]):
                        new.append(mybir.InstNoOp(
                            name=f"{inst.name}-wsplit{j}",
                            sync_info=mybir.SyncInfo(on_wait=[w], on_update=[]),
                            engine=inst.engine,
                            bass_nofuse=True,
                        ))
                        n += 1
                    si.on_wait = waits[:1]
                    dirty = True
                new.append(inst)
            if dirty:
                bb.instructions = new
    return n


def _build(with_mask, split=True, cfg=None):
    cfg = {**_DEFAULT_CFG, **(cfg or {})}
    nc = bass.Bass("TRN2")
    f32r = dt.float32r
    f32 = dt.float32

    xt = nc.dram_tensor("xt", [C, S], f32r, kind="ExternalInput")
    wqt = nc.dram_tensor("wqt", [C, GC], f32r, kind="ExternalInput")
    wkt = nc.dram_tensor("wkt", [C, GC], f32r, kind="ExternalInput")
    wvt = nc.dram_tensor("wvt", [C, GC], f32r, kind="ExternalInput")
    wpt = nc.dram_tensor("wpt", [GC, C], f32r, kind="ExternalInput")
    bqd = nc.dram_tensor("bqd", [GC], f32, kind="ExternalInput")
    bkd = nc.dram_tensor("bkd", [GC], f32, kind="ExternalInput")
    onesd = nc.dram_tensor("onesd", [1, D], f32r, kind="ExternalInput")
    vonesd = nc.dram_tensor("vonesd", [128, NSB * 8], f32r, kind="ExternalInput")
    dmaskd = nc.dram_tensor("dmaskd", [128, 128], f32, kind="ExternalInput")
    if with_mask:
        masktd = nc.dram_tensor("masktd", [S, S], f32, kind="ExternalInput")
    outd = nc.dram_tensor("out", [S, C], f32, kind="ExternalOutput")

    with tile.TileContext(nc) as tc:
        with tc.tile_pool(name="persist", bufs=1) as persist:
            qt_sb = [persist.tile([128, S], f32r, tag=f"qt{t}", name=f"qt{t}")
                     for t in range(NPAIR)]
            kt_sb = [persist.tile([128, S], f32r, tag=f"kt{t}", name=f"kt{t}")
                     for t in range(NPAIR)]
            v_sb = persist.tile([128, NSB * 520], f32r, tag="v")
            ones_sb = persist.tile([1, D], f32r, tag="ones")
            nc.sync.dma_start(out=ones_sb, in_=onesd[:, :])
            bq_sb = persist.tile([128, 4], f32, tag="bq")
            nc.sync.dma_start(out=bq_sb, in_=bqd.rearrange("(t p) -> p t", p=128))
            bk_sb = persist.tile([128, 4], f32, tag="bk")
            nc.sync.dma_start(out=bk_sb, in_=bkd.rearrange("(t p) -> p t", p=128))
            # ones columns of v (position 64 of each head's 65-col slot)
            nc.sync.dma_start(
                out=v_sb.rearrange("p (s h d) -> p s h d", h=8, d=65)[:, :, :, 64:65],
                in_=vonesd.rearrange("p (s h) -> p s h", h=8).unsqueeze(3),
            )

            # ---------------- phase 1: projections ----------------
            with tc.tile_pool(name="xtp", bufs=1) as xtp, \
                 tc.tile_pool(name="wqk", bufs=cfg["wqk_bufs"]) as wqk, \
                 tc.tile_pool(name="wvp", bufs=1) as wvp, \
                 tc.tile_pool(name="pp", bufs=cfg["pp_bufs"], space="PSUM") as pp:
                xt_t = xtp.tile([128, NCI, S], f32r)
                nc.sync.dma_start(out=xt_t, in_=xt.rearrange("(k p) s -> p k s", p=128))

                for t in range(NPAIR):
                    for name, wsrc, dst, bias in (
                        ("q", wqt, qt_sb[t], bq_sb),
                        ("k", wkt, kt_sb[t], bk_sb),
                    ):
                        w_t = wqk.tile([128, NCI, 128], f32r, tag="w")
                        nc.sync.dma_start(
                            out=w_t,
                            in_=wsrc[:, 128 * t:128 * t + 128]
                                .rearrange("(k p) c -> p k c", p=128))
                        for ch in range(NCH):
                            ps = pp.tile([128, 512], dt.float32, tag="ps")
                            for ci in range(NCI):
                                nc.tensor.matmul(
                                    ps, w_t[:, ci, :],
                                    xt_t[:, ci, 512 * ch:512 * ch + 512],
                                    start=(ci == 0), stop=(ci == NCI - 1))
                            if name == "q":
                                nc.vector.tensor_scalar(
                                    dst[:, 512 * ch:512 * ch + 512], ps,
                                    SCALE, bias[:, t:t + 1],
                                    op0=Alu.mult, op1=Alu.add)
                            else:
                                nc.vector.tensor_scalar(
                                    dst[:, 512 * ch:512 * ch + 512], ps,
                                    bias[:, t:t + 1], None, op0=Alu.add)

                wv_t = wvp.tile([128, NCI, GC], f32r)
                nc.sync.dma_start(out=wv_t, in_=wvt.rearrange("(k p) c -> p k c", p=128))
                v3 = v_sb.rearrange("p (s h) -> p s h", h=520)
                for sb in range(NSB):
                    ps = pp.tile([128, GC], dt.float32, tag="ps")
                    for ci in range(NCI):
                        nc.tensor.matmul(
                            ps, xt_t[:, ci, 128 * sb:128 * sb + 128],
                            wv_t[:, ci, :],
                            start=(ci == 0), stop=(ci == NCI - 1))
                    nc.vector.tensor_copy(
                        out=v3[:, sb, :].rearrange("p (h d) -> p h d", d=65)[:, :, 0:64],
                        in_=ps.rearrange("p (h d) -> p h d", d=64))

            # ---------------- phases 2+3 ----------------
            with tc.tile_pool(name="ph23", bufs=1) as ph23, \
                 tc.tile_pool(name="mstr", bufs=3) as mstr, \
                 tc.tile_pool(name="pt", bufs=cfg["pt_bufs"]) as pt, \
                 tc.tile_pool(name="nrm", bufs=cfg["nrm_bufs"]) as nrm:
                y_sb = [ph23.tile([128, S], f32r, tag=f"y{t}", name=f"y{t}")
                        for t in range(NPAIR)]
                dm_sb = ph23.tile([128, 128], dt.float32, tag="dm")
                nc.sync.dma_start(out=dm_sb, in_=dmaskd[:, :])

                # phase 2: attention, chunk-outer so the chunk's output
                # projection overlaps the next chunk's attention
                with tc.tile_pool(name="wpp", bufs=1) as wpp, \
                     tc.tile_pool(name="ost", bufs=cfg["ost_bufs"]) as ost, \
                     tc.tile_pool(name="ps2", bufs=cfg["sc_bufs"], space="PSUM") as ps2, \
                     tc.tile_pool(name="py2", bufs=cfg["py_bufs"], space="PSUM") as py2, \
                     tc.tile_pool(name="pr2", bufs=cfg["rep_bufs"], space="PSUM") as pr2, \
                     tc.tile_pool(name="po3", bufs=cfg["po_bufs"], space="PSUM") as po3:
                    wp_t = wpp.tile([128, NPAIR, C], f32r)
                    nc.sync.dma_start(out=wp_t,
                                      in_=wpt.rearrange("(t p) c -> p t c", p=128))
                    for ch in range(NCH):
                        for t in range(NPAIR):
                            nkb = 4 * (ch + 1)
                            py = [py2.tile([65, 512], dt.float32, tag="py", name=f"py{_h}")
                                  for _h in range(2)]
                            for kb in range(nkb):
                                # valid q-range for this k-block: only
                                # columns q >= 128*kb have unmasked scores;
                                # everything below is skipped outright (the
                                # AV accumulation simply never touches them).
                                off = max(0, 128 * kb - 512 * ch)
                                w = 512 - off
                                ps = ps2.tile([128, 1024], dt.float32, tag="sc")
                                for h in range(2):
                                    nc.tensor.matmul(
                                        ps[:, 512 * h + off:512 * h + 512],
                                        kt_sb[t][64 * h:64 * h + 64,
                                                 128 * kb:128 * kb + 128],
                                        qt_sb[t][64 * h:64 * h + 64,
                                                 512 * ch + off:512 * ch + 512],
                                        start=True, stop=True,
                                        tile_position=(64 * h, 0))
                                if kb >= 4 * ch and not cfg.get("no_dmask"):
                                    # triangle mask on the 128-wide diag block
                                    for h in range(2):
                                        nc.vector.tensor_tensor(
                                            out=ps[:, 512 * h + off:
                                                   512 * h + off + 128],
                                            in0=ps[:, 512 * h + off:
                                                   512 * h + off + 128],
                                            in1=dm_sb, op=Alu.add)
                                if with_mask:
                                    mt = mstr.tile([128, 512], dt.float32, tag="mt")
                                    nc.sync.dma_start(
                                        out=mt[:, 0:w],
                                        in_=masktd[128 * kb:128 * kb + 128,
                                                   512 * ch + off:512 * ch + 512])
                                    for h in range(2):
                                        nc.vector.tensor_tensor(
                                            out=ps[:, 512 * h + off:512 * h + 512],
                                            in0=ps[:, 512 * h + off:512 * h + 512],
                                            in1=mt[:, 0:w], op=Alu.add)
                                p_sb = pt.tile([128, 1024], f32r, tag="p")
                                ps3 = ps.rearrange("p (h w) -> p h w", h=2)
                                pb3 = p_sb.rearrange("p (h w) -> p h w", h=2)
                                nc.scalar.activation(pb3[:, :, off:512],
                                                     ps3[:, :, off:512], F.Exp)
                                for h in range(2):
                                    nc.tensor.matmul(
                                        py[h][:, off:512],
                                        v_sb[:, 520 * kb + 65 * (2 * t + h):
                                             520 * kb + 65 * (2 * t + h) + 65],
                                        p_sb[:, 512 * h + off:512 * h + 512],
                                        start=(kb == 0), stop=(kb == nkb - 1))
                            # normalize: y = py[0:64] * broadcast(1/py[64])
                            if cfg.get("no_norm"):
                                for h in range(2):
                                    if h == 0:
                                        nc.vector.tensor_copy(
                                            out=y_sb[t][0:64, 512 * ch:512 * ch + 512],
                                            in_=py[h][0:64, :])
                                    else:
                                        ytmp = nrm.tile([64, 512], f32r, tag="ytmp")
                                        nc.vector.tensor_copy(out=ytmp, in_=py[h][0:64, :])
                                        nc.sync.dma_start(
                                            out=y_sb[t][64:128, 512 * ch:512 * ch + 512],
                                            in_=ytmp)
                                continue
                            for h in range(2):
                                rec = nrm.tile([1, 512], f32r, tag="rec")
                                with nc.allow_low_precision(reason="fp32r recip"):
                                    nc.vector.reciprocal(rec, py[h][64:65, :])
                                rep = pr2.tile([64, 512], dt.float32, tag="rep")
                                nc.tensor.matmul(rep, ones_sb, rec,
                                                 start=True, stop=True)
                                rep_sb = nrm.tile([64, 512], dt.float32, tag="repsb")
                                nc.vector.tensor_copy(rep_sb, rep)
                                if h == 0:
                                    nc.vector.tensor_tensor(
                                        out=y_sb[t][0:64, 512 * ch:512 * ch + 512],
                                        in0=py[h][0:64, :], in1=rep_sb, op=Alu.mult)
                                else:
                                    ytmp = nrm.tile([64, 512], f32r, tag="ytmp")
                                    nc.vector.tensor_tensor(
                                        out=ytmp, in0=py[h][0:64, :],
                                        in1=rep_sb, op=Alu.mult)
                                    nc.sync.dma_start(
                                        out=y_sb[t][64:128, 512 * ch:512 * ch + 512],
                                        in_=ytmp)

                        # output projection for this chunk's s-rows
                        if cfg.get("no_p3"):
                            continue
                        for sbl in range(4):
                            sb = 4 * ch + sbl
                            for j in range(2):
                                po = po3.tile([128, 512], dt.float32, tag="po")
                                for t in range(NPAIR):
                                    # K=128: the pair's two heads stack to a
                                    # full contraction, no row-split needed
                                    # (row-split accumulation groups hang
                                    # this HW/compiler).
                                    nc.tensor.matmul(
                                        po,
                                        y_sb[t][:, 128 * sb:128 * sb + 128],
                                        wp_t[:, t, 512 * j:512 * j + 512],
                                        start=(t == 0),
                                        stop=(t == NPAIR - 1))
                                o_sb = ost.tile([128, 512], dt.float32, tag="o")
                                nc.vector.tensor_copy(o_sb, po)
                                nc.sync.dma_start(
                                    out=outd[128 * sb:128 * sb + 128,
                                             512 * j:512 * j + 512],
                                    in_=o_sb)

    if split:
        _split_multi_waits(nc)
    return nc


def _get_nc(with_mask):
    if with_mask not in _nc_cache:
        _nc_cache[with_mask] = _build(with_mask)
    return _nc_cache[with_mask]


def _host_consts():
    ones = np.ones((1, D), np.float32)
    vones = np.ones((128, NSB * 8), np.float32)
    i, j = np.meshgrid(np.arange(128), np.arange(128), indexing="ij")
    dm = np.where(j < i, np.float32(NEG), np.float32(0.0))
    return ones, vones, dm.astype(np.float32)


def make_in_maps(tgt, pad_mask, Wq, bq, Wk, bk, Wv, bv, Wp, bp, with_mask):
    ones, vones, dm = _host_consts()
    in_maps = []
    for core in range(8):
        b, g = core // 2, core % 2
        rows = slice(GC * g, GC * g + GC)
        im = {
            "xt": np.ascontiguousarray(tgt[b].T),
            "wqt": np.ascontiguousarray(Wq[rows].T),
            "wkt": np.ascontiguousarray(Wk[rows].T),
            "wvt": np.ascontiguousarray(Wv[rows].T),
            "wpt": np.ascontiguousarray(Wp[:, rows].T),
            "bqd": np.ascontiguousarray(bq[rows] * SCALE),
            "bkd": np.ascontiguousarray(bk[rows]),
            "onesd": ones,
            "vonesd": vones,
            "dmaskd": dm,
        }
        if with_mask:
            im["masktd"] = np.ascontiguousarray(pad_mask[b].T)
        in_maps.append(im)
    return in_maps


def run(tgt, pad_mask, Wq, bq, Wk, bk, Wv, bv, Wp, bp, **spmd_kwargs):
    args = [np.asarray(a, np.float32) for a in
            (tgt, pad_mask, Wq, bq, Wk, bk, Wv, bv, Wp, bp)]
    tgt, pad_mask, Wq, bq, Wk, bk, Wv, bv, Wp, bp = args
    with_mask = bool(np.any(pad_mask))
    nc = _get_nc(with_mask)
    in_maps = make_in_maps(tgt, pad_mask, Wq, bq, Wk, bk, Wv, bv, Wp, bp,
                           with_mask)
    res = run_bass_kernel_spmd(nc, in_maps, core_ids=list(range(8)),
                               **spmd_kwargs)
    bias_vec = (bp + Wp @ bv).astype(np.float32)
    out = np.empty((B, S, C), np.float32)
    for b in range(B):
        out[b] = (res.results[2 * b]["out"] + res.results[2 * b + 1]["out"]
                  + bias_vec)
    return out, res


def kernel(tgt, pad_mask, Wq, bq, Wk, bk, Wv, bv, Wp, bp):
    out, _ = run(tgt, pad_mask, Wq, bq, Wk, bk, Wv, bv, Wp, bp)
    return out


# revision 20
# speedup vs baseline: 20.2730x; 1.0926x over previous
"""Causal self-attention (B=4, S=2048, C=1024, H=16) on 8 TRN2 cores.

Sharding: core = (batch b = core//2, head-group g = core%2). Each core
computes q/k/v projections for its 8 heads, causal attention, and a
partial output projection; the host sums the two per-batch partials and
adds the (host-computed) bias vector bp + Wp @ bv.

On-chip layout is transpose-free: the host passes tgt[b].T and the
weight slices pre-transposed, so every matmul operand already has the
contraction dim on partitions. All matmuls run as float32r (measured
~1e-5 rel-rms on HW, full rate at N>=256).

Softmax runs without max-subtraction (scores are bounded ~|20| for this
problem's 0.02-scale weights; fp32 exp is safe to ~88). Causality is
enforced with an additive -1e10 block/triangle mask fused into the
score PSUM; the denominator comes from a ones-column appended to V so
the AV matmul yields [y; sum(p)] in one pass.
"""
import numpy as np

import concourse.bass as bass
import concourse.mybir as mybir
import concourse.tile as tile
from concourse.bass_utils import run_bass_kernel_spmd

dt = mybir.dt
F = mybir.ActivationFunctionType
Alu = mybir.AluOpType

B, S, C, H = 4, 2048, 1024, 16
D = C // H            # 64 head dim
GC = C // 2           # 512 channels per head-group (8 heads)
NPAIR = 4             # head pairs per core
NCH = S // 512        # 4 query chunks
NSB = S // 128        # 16 s blocks
NCI = C // 128        # 8 contraction blocks
SCALE = 0.125         # 1/sqrt(D)
NEG = -1.0e10

_nc_cache = {}

_DEFAULT_CFG = {
    'pt_bufs': 4,
    'sc_bufs': 2,
    'py_bufs': 2,
    'rep_bufs': 1,
    'pp_bufs': 6,
    'wqk_bufs': 2,
    'ost_bufs': 2,
    'po_bufs': 1,
    'nrm_bufs': 2,
}


def _split_multi_waits(nc):
    """This container's walrus accepts at most ONE sem wait per
    instruction ("Too many sync wait commands"). Hoist extra waits onto
    NOPs inserted just before the instruction on the same engine."""
    n = 0
    for fn in nc.m.functions:
        for bb in fn.blocks:
            new = []
            dirty = False
            for inst in bb.instructions:
                si = inst.sync_info
                if si is not None and si.on_wait and len(si.on_wait) > 1:
                    waits = list(si.on_wait)
                    for j, w in enumerate(waits[1:]):
                        new.append(mybir.InstNoOp(
                            name=f"{inst.name}-wsplit{j}",
                            sync_info=mybir.SyncInfo(on_wait=[w], on_update=[]),
                            engine=inst.engine,
                            bass_nofuse=True,
                        ))
                        n += 1
                    si.on_wait = waits[:1]
                    dirty = True
                new.append(inst)
            if dirty:
                bb.instructions = new
    return n


def _build(with_mask, split=True, cfg=None):
    cfg = {**_DEFAULT_CFG, **(cfg or {})}
    nc = bass.Bass("TRN2")
    f32r = dt.float32r
    f32 = dt.float32

    xt = nc.dram_tensor("xt", [C, S], f32r, kind="ExternalInput")
    wqt = nc.dram_tensor("wqt", [C, GC], f32r, kind="ExternalInput")
    wkt = nc.dram_tensor("wkt", [C, GC], f32r, kind="ExternalInput")
    wvt = nc.dram_tensor("wvt", [C, GC], f32r, kind="ExternalInput")
    wpt = nc.dram_tensor("wpt", [GC, C], f32r, kind="ExternalInput")
    bqd = nc.dram_tensor("bqd", [GC], f32, kind="ExternalInput")
    bkd = nc.dram_tensor("bkd", [GC], f32, kind="ExternalInput")
    onesd = nc.dram_tensor("onesd", [1, D], f32r, kind="ExternalInput")
    vonesd = nc.dram_tensor("vonesd", [128, NSB * 8], f32r, kind="ExternalInput")
    dmaskd = nc.dram_tensor("dmaskd", [128, 256], f32, kind="ExternalInput")
    if with_mask:
        masktd = nc.dram_tensor("masktd", [S, S], f32, kind="ExternalInput")
    outd = nc.dram_tensor("out", [S, C], f32, kind="ExternalOutput")

    with tile.TileContext(nc) as tc:
        with tc.tile_pool(name="persist", bufs=1) as persist:
            qt_sb = [persist.tile([128, S], f32r, tag=f"qt{t}", name=f"qt{t}")
                     for t in range(NPAIR)]
            kt_sb = [persist.tile([128, S], f32r, tag=f"kt{t}", name=f"kt{t}")
                     for t in range(NPAIR)]
            v_sb = persist.tile([128, NSB * 520], f32r, tag="v")
            ones_sb = persist.tile([1, D], f32r, tag="ones")
            nc.sync.dma_start(out=ones_sb, in_=onesd[:, :])
            bq_sb = persist.tile([128, 4], f32, tag="bq")
            nc.sync.dma_start(out=bq_sb, in_=bqd.rearrange("(t p) -> p t", p=128))
            bk_sb = persist.tile([128, 4], f32, tag="bk")
            nc.sync.dma_start(out=bk_sb, in_=bkd.rearrange("(t p) -> p t", p=128))
            # ones columns of v (position 64 of each head's 65-col slot)
            nc.sync.dma_start(
                out=v_sb.rearrange("p (s h d) -> p s h d", h=8, d=65)[:, :, :, 64:65],
                in_=vonesd.rearrange("p (s h) -> p s h", h=8).unsqueeze(3),
            )

            # ---------------- phase 1: projections ----------------
            with tc.tile_pool(name="xtp", bufs=1) as xtp, \
                 tc.tile_pool(name="wqk", bufs=cfg["wqk_bufs"]) as wqk, \
                 tc.tile_pool(name="wvp", bufs=1) as wvp, \
                 tc.tile_pool(name="pp", bufs=cfg["pp_bufs"], space="PSUM") as pp:
                xt_t = xtp.tile([128, NCI, S], f32r)
                nc.sync.dma_start(out=xt_t, in_=xt.rearrange("(k p) s -> p k s", p=128))

                for t in range(NPAIR):
                    for name, wsrc, dst, bias in (
                        ("q", wqt, qt_sb[t], bq_sb),
                        ("k", wkt, kt_sb[t], bk_sb),
                    ):
                        w_t = wqk.tile([128, NCI, 128], f32r, tag="w")
                        nc.sync.dma_start(
                            out=w_t,
                            in_=wsrc[:, 128 * t:128 * t + 128]
                                .rearrange("(k p) c -> p k c", p=128))
                        for ch in range(NCH):
                            ps = pp.tile([128, 512], dt.float32, tag="ps")
                            for ci in range(NCI):
                                nc.tensor.matmul(
                                    ps, w_t[:, ci, :],
                                    xt_t[:, ci, 512 * ch:512 * ch + 512],
                                    start=(ci == 0), stop=(ci == NCI - 1))
                            if name == "q":
                                nc.vector.tensor_scalar(
                                    dst[:, 512 * ch:512 * ch + 512], ps,
                                    SCALE, bias[:, t:t + 1],
                                    op0=Alu.mult, op1=Alu.add)
                            else:
                                nc.vector.tensor_scalar(
                                    dst[:, 512 * ch:512 * ch + 512], ps,
                                    bias[:, t:t + 1], None, op0=Alu.add)

                wv_t = wvp.tile([128, NCI, GC], f32r)
                nc.sync.dma_start(out=wv_t, in_=wvt.rearrange("(k p) c -> p k c", p=128))
                v3 = v_sb.rearrange("p (s h) -> p s h", h=520)
                for sb in range(NSB):
                    ps = pp.tile([128, GC], dt.float32, tag="ps")
                    for ci in range(NCI):
                        nc.tensor.matmul(
                            ps, xt_t[:, ci, 128 * sb:128 * sb + 128],
                            wv_t[:, ci, :],
                            start=(ci == 0), stop=(ci == NCI - 1))
                    nc.vector.tensor_copy(
                        out=v3[:, sb, :].rearrange("p (h d) -> p h d", d=65)[:, :, 0:64],
                        in_=ps.rearrange("p (h d) -> p h d", d=64))

            # ---------------- phases 2+3 ----------------
            with tc.tile_pool(name="ph23", bufs=1) as ph23, \
                 tc.tile_pool(name="mstr", bufs=3) as mstr, \
                 tc.tile_pool(name="pt", bufs=cfg["pt_bufs"]) as pt, \
                 tc.tile_pool(name="nrm", bufs=cfg["nrm_bufs"]) as nrm:
                y_sb = [ph23.tile([128, S], f32r, tag=f"y{t}", name=f"y{t}")
                        for t in range(NPAIR)]
                dm_sb = ph23.tile([128, 256], dt.float32, tag="dm")
                nc.sync.dma_start(out=dm_sb, in_=dmaskd[:, :])

                # phase 2: attention, chunk-outer so the chunk's output
                # projection overlaps the next chunk's attention
                with tc.tile_pool(name="wpp", bufs=1) as wpp, \
                     tc.tile_pool(name="ost", bufs=cfg["ost_bufs"]) as ost, \
                     tc.tile_pool(name="ps2", bufs=cfg["sc_bufs"], space="PSUM") as ps2, \
                     tc.tile_pool(name="py2", bufs=cfg["py_bufs"], space="PSUM") as py2, \
                     tc.tile_pool(name="pr2", bufs=cfg["rep_bufs"], space="PSUM") as pr2, \
                     tc.tile_pool(name="po3", bufs=cfg["po_bufs"], space="PSUM") as po3:
                    wp_t = wpp.tile([128, NPAIR, C], f32r)
                    nc.sync.dma_start(out=wp_t,
                                      in_=wpt.rearrange("(t p) c -> p t c", p=128))
                    for ch in range(NCH):
                        for t in range(NPAIR):
                            nkb = 4 * (ch + 1)
                            py = [py2.tile([65, 512], dt.float32, tag="py", name=f"py{_h}")
                                  for _h in range(2)]
                            for kb in range(nkb):
                                # valid q-range for this k-block: only
                                # columns q >= 128*kb have unmasked scores;
                                # everything below is skipped outright (the
                                # AV accumulation simply never touches them).
                                off = max(0, 128 * kb - 512 * ch)
                                w = 512 - off
                                ps = ps2.tile([128, 1024], dt.float32, tag="sc")
                                for h in range(2):
                                    nc.tensor.matmul(
                                        ps[:, 512 * h + off:512 * h + 512],
                                        kt_sb[t][64 * h:64 * h + 64,
                                                 128 * kb:128 * kb + 128],
                                        qt_sb[t][64 * h:64 * h + 64,
                                                 512 * ch + off:512 * ch + 512],
                                        start=True, stop=True,
                                        tile_position=(64 * h, 0))
                                diag = kb >= 4 * ch and not cfg.get("no_dmask")
                                if diag and with_mask:
                                    # additive triangle on the scores (the
                                    # pad-mask path adds to PSUM anyway)
                                    for h in range(2):
                                        nc.vector.tensor_tensor(
                                            out=ps[:, 512 * h + off:
                                                   512 * h + off + 128],
                                            in0=ps[:, 512 * h + off:
                                                   512 * h + off + 128],
                                            in1=dm_sb[:, 0:128], op=Alu.add)
                                if with_mask:
                                    mt = mstr.tile([128, 512], dt.float32, tag="mt")
                                    nc.sync.dma_start(
                                        out=mt[:, 0:w],
                                        in_=masktd[128 * kb:128 * kb + 128,
                                                   512 * ch + off:512 * ch + 512])
                                    for h in range(2):
                                        nc.vector.tensor_tensor(
                                            out=ps[:, 512 * h + off:512 * h + 512],
                                            in0=ps[:, 512 * h + off:512 * h + 512],
                                            in1=mt[:, 0:w], op=Alu.add)
                                p_sb = pt.tile([128, 1024], f32r, tag="p")
                                ps3 = ps.rearrange("p (h w) -> p h w", h=2)
                                pb3 = p_sb.rearrange("p (h w) -> p h w", h=2)
                                nc.scalar.activation(pb3[:, :, off:512],
                                                     ps3[:, :, off:512], F.Exp)
                                if diag and not with_mask:
                                    # zero the causally-invalid triangle of p
                                    # AFTER exp (SBUF multiply by 0/1), so
                                    # exp never waits on the DVE
                                    for h in range(2):
                                        nc.vector.tensor_tensor(
                                            out=p_sb[:, 512 * h + off:
                                                     512 * h + off + 128],
                                            in0=p_sb[:, 512 * h + off:
                                                     512 * h + off + 128],
                                            in1=dm_sb[:, 128:256].bitcast(
                                                dt.float32r), op=Alu.mult)
                                for h in range(2):
                                    nc.tensor.matmul(
                                        py[h][:, off:512],
                                        v_sb[:, 520 * kb + 65 * (2 * t + h):
                                             520 * kb + 65 * (2 * t + h) + 65],
                                        p_sb[:, 512 * h + off:512 * h + 512],
                                        start=(kb == 0), stop=(kb == nkb - 1))
                            # normalize: y = py[0:64] * broadcast(1/py[64])
                            if cfg.get("no_norm"):
                                for h in range(2):
                                    if h == 0:
                                        nc.vector.tensor_copy(
                                            out=y_sb[t][0:64, 512 * ch:512 * ch + 512],
                                            in_=py[h][0:64, :])
                                    else:
                                        ytmp = nrm.tile([64, 512], f32r, tag="ytmp")
                                        nc.vector.tensor_copy(out=ytmp, in_=py[h][0:64, :])
                                        nc.sync.dma_start(
                                            out=y_sb[t][64:128, 512 * ch:512 * ch + 512],
                                            in_=ytmp)
                                continue
                            for h in range(2):
                                rec = nrm.tile([1, 512], f32r, tag="rec")
                                with nc.allow_low_precision(reason="fp32r recip"):
                                    nc.vector.reciprocal(rec, py[h][64:65, :])
                                rep = pr2.tile([64, 512], dt.float32, tag="rep")
                                nc.tensor.matmul(rep, ones_sb, rec,
                                                 start=True, stop=True)
                                rep_sb = nrm.tile([64, 512], dt.float32, tag="repsb")
                                nc.vector.tensor_copy(rep_sb, rep)
                                if h == 0:
                                    nc.vector.tensor_tensor(
                                        out=y_sb[t][0:64, 512 * ch:512 * ch + 512],
                                        in0=py[h][0:64, :], in1=rep_sb, op=Alu.mult)
                                else:
                                    ytmp = nrm.tile([64, 512], f32r, tag="ytmp")
                                    nc.vector.tensor_tensor(
                                        out=ytmp, in0=py[h][0:64, :],
                                        in1=rep_sb, op=Alu.mult)
                                    nc.sync.dma_start(
                                        out=y_sb[t][64:128, 512 * ch:512 * ch + 512],
                                        in_=ytmp)

                        # output projection for this chunk's s-rows
                        if cfg.get("no_p3"):
                            continue
                        for sbl in range(4):
                            sb = 4 * ch + sbl
                            for j in range(2):
                                po = po3.tile([128, 512], dt.float32, tag="po")
                                for t in range(NPAIR):
                                    # K=128: the pair's two heads stack to a
                                    # full contraction, no row-split needed
                                    # (row-split accumulation groups hang
                                    # this HW/compiler).
                                    nc.tensor.matmul(
                                        po,
                                        y_sb[t][:, 128 * sb:128 * sb + 128],
                                        wp_t[:, t, 512 * j:512 * j + 512],
                                        start=(t == 0),
                                        stop=(t == NPAIR - 1))
                                o_sb = ost.tile([128, 512], dt.float32, tag="o")
                                nc.vector.tensor_copy(o_sb, po)
                                nc.sync.dma_start(
                                    out=outd[128 * sb:128 * sb + 128,
                                             512 * j:512 * j + 512],
                                    in_=o_sb)

    if split:
        _split_multi_waits(nc)
    return nc


def _get_nc(with_mask):
    if with_mask not in _nc_cache:
        _nc_cache[with_mask] = _build(with_mask)
    return _nc_cache[with_mask]


def _host_consts():
    ones = np.ones((1, D), np.float32)
    vones = np.ones((128, NSB * 8), np.float32)
    i, j = np.meshgrid(np.arange(128), np.arange(128), indexing="ij")
    dm = np.concatenate([
        np.where(j < i, np.float32(NEG), np.float32(0.0)),   # additive
        np.where(j < i, np.float32(0.0), np.float32(1.0)),   # 0/1 multiply
    ], axis=1)
    return ones, vones, dm.astype(np.float32)


def make_in_maps(tgt, pad_mask, Wq, bq, Wk, bk, Wv, bv, Wp, bp, with_mask):
    ones, vones, dm = _host_consts()
    in_maps = []
    for core in range(8):
        b, g = core // 2, core % 2
        rows = slice(GC * g, GC * g + GC)
        im = {
            "xt": np.ascontiguousarray(tgt[b].T),
            "wqt": np.ascontiguousarray(Wq[rows].T),
            "wkt": np.ascontiguousarray(Wk[rows].T),
            "wvt": np.ascontiguousarray(Wv[rows].T),
            "wpt": np.ascontiguousarray(Wp[:, rows].T),
            "bqd": np.ascontiguousarray(bq[rows] * SCALE),
            "bkd": np.ascontiguousarray(bk[rows]),
            "onesd": ones,
            "vonesd": vones,
            "dmaskd": dm,
        }
        if with_mask:
            im["masktd"] = np.ascontiguousarray(pad_mask[b].T)
        in_maps.append(im)
    return in_maps


def run(tgt, pad_mask, Wq, bq, Wk, bk, Wv, bv, Wp, bp, **spmd_kwargs):
    args = [np.asarray(a, np.float32) for a in
            (tgt, pad_mask, Wq, bq, Wk, bk, Wv, bv, Wp, bp)]
    tgt, pad_mask, Wq, bq, Wk, bk, Wv, bv, Wp, bp = args
    with_mask = bool(np.any(pad_mask))
    nc = _get_nc(with_mask)
    in_maps = make_in_maps(tgt, pad_mask, Wq, bq, Wk, bk, Wv, bv, Wp, bp,
                           with_mask)
    res = run_bass_kernel_spmd(nc, in_maps, core_ids=list(range(8)),
                               **spmd_kwargs)
    bias_vec = (bp + Wp @ bv).astype(np.float32)
    out = np.empty((B, S, C), np.float32)
    for b in range(B):
        out[b] = (res.results[2 * b]["out"] + res.results[2 * b + 1]["out"]
                  + bias_vec)
    return out, res


def kernel(tgt, pad_mask, Wq, bq, Wk, bk, Wv, bv, Wp, bp):
    out, _ = run(tgt, pad_mask, Wq, bq, Wk, bk, Wv, bv, Wp, bp)
    return out
